# revision 1
# baseline (speedup 1.0000x reference)
"""Trainium2 Bass kernel for MockMobGatedDeltaNetMoE.

Sharding: head-parallel over H=8 heads, one head per NeuronCore.
Each core computes its head's full contribution (projections, routing,
ratio-expert attention, gated combine, output projection partial-sum);
the host sums the 8 partial outputs.

Math notes (exact-equivalent reformulations of the reference):
 - softmax(x) ratios computed from exp(x) directly (no max-subtract; logits
   are ~N(0,1) so exp is safe in fp32).
 - router: top-2 of 4 via two reduce_max passes; weights s_i/(2*(m1+m2)).
 - attention: masked keys contribute exp(0)=1 to the denominator and 0 to
   the numerator. We compute exp(S/16 - 30*(1-m_k)) (masked keys -> ~1e-13),
   and add back cnt = #masked keys to the denominator via a ones-matmul.
 - per-(r,q) combine scalar c = rw / denom folded into PSUM eviction.
All matmuls run as float32r (full fp32 data, fast PE mode).
"""

import numpy as np

import concourse.bass as bass
import concourse.bacc as bacc
import concourse.tile as tile
from concourse import mybir
from concourse.bass_utils import run_bass_kernel_spmd

F32 = mybir.dt.float32
F32R = mybir.dt.float32r
BF16 = mybir.dt.bfloat16
ALU = mybir.AluOpType
ACTF = mybir.ActivationFunctionType
AX = mybir.AxisListType

H, D, R, NE = 8, 256, 6, 4          # heads, head_dim, experts, routed experts
HID, DV, T = 2048, 512, 2048        # hidden, head_v_dim, b*t tokens
NB = 2                              # batch
TB = T // NB                        # tokens per batch (attention window)
SCALE = 1.0 / 16.0                  # 1/sqrt(D)
NEG = -30.0                         # masked-key logit bias


def _r(ap):
    return ap


def _body(ctx, nc, tc, io):
    hsT, wq, wk, wv, wg, wqe, wke, hsh, hsl, wfh, wfl, wo, out = io
    import os
    SKIP = set(os.environ.get("KSKIP", "").split(","))

    import contextlib

    const = ctx.enter_context(tc.tile_pool(name="const", bufs=1))
    pers = ctx.enter_context(tc.tile_pool(name="pers", bufs=1))

    ones_f32 = const.tile([128, 128], F32, name="ones_f32")
    nc.vector.memset(ones_f32[:], 1.0)
    ones2 = const.tile([128, 2], F32R, name="ones2")
    nc.scalar.copy(ones2[:], ones_f32[:, 0:2])
    ones128 = const.tile([128, 128], F32R, name="ones128")
    nc.scalar.copy(ones128[:], ones_f32[:])
    from concourse.masks import make_identity
    ident = const.tile([128, 128], F32, name="ident")
    make_identity(nc, ident)
    # fused routing weight (Wq_head @ Wgate, host-fp64) split hi/lo bf16
    wfh_sb = const.tile([128, 64], BF16, name="wfh_sb")
    wfl_sb = const.tile([128, 64], BF16, name="wfl_sb")
    for hc in range(16):
        nc.sync.dma_start(out=wfh_sb[:, hc * 4:(hc + 1) * 4],
                          in_=wfh[hc * 128:(hc + 1) * 128, :])
        nc.sync.dma_start(out=wfl_sb[:, hc * 4:(hc + 1) * 4],
                          in_=wfl[hc * 128:(hc + 1) * 128, :])
    logit_sb = pers.tile([128, 64], F32, name="logit_sb")

    # persistent tensors (col-blocked single tiles)
    qT = pers.tile([128, 2 * T], F32R, name="qT")        # [d-chunk, token]
    kT = pers.tile([128, 2 * T], F32R, name="kT")
    wqe_sb = pers.tile([128, 2 * 1536], F32R, name="wqe_sb")
    wke_sb = pers.tile([128, 2 * 1536], F32R, name="wke_sb")
    rw_all = pers.tile([128, 16 * R], F32, name="rw_all")
    biasN = pers.tile([128, 16 * NE], F32, name="biasN")
    invm = pers.tile([128, 16 * NE], F32R, name="invm")
    for dc in range(2):
        nc.sync.dma_start(out=wqe_sb[:, dc * 1536:(dc + 1) * 1536],
                          in_=wqe[dc * 128:(dc + 1) * 128, :])
        nc.sync.dma_start(out=wke_sb[:, dc * 1536:(dc + 1) * 1536],
                          in_=wke[dc * 128:(dc + 1) * 128, :])

    dram = ctx.enter_context(tc.tile_pool(name="dram", bufs=1, space="DRAM"))
    v_dram = dram.tile([T, DV], F32R, name="v_dram")
    g_dram = dram.tile([T, DV], F32, name="g_dram")

    # ---------------- phase 1: projections ----------------
    with tc.tile_pool(name="p1", bufs=1) as p1, \
         tc.tile_pool(name="p1ps", bufs=1, space="PSUM") as p1ps:
        for tb in range(4):  # token blocks of 512
            t0 = tb * 512
            hst = []
            for hc in range(16):
                ht = p1.tile([128, 512], F32R, name="hst", tag="hst", bufs=17)
                nc.sync.dma_start(out=ht[:], in_=hsT[hc * 128:(hc + 1) * 128, t0:t0 + 512])
                hst.append(ht)
            for wsrc, dstT in (() if "p1q" in SKIP else ((wq, qT), (wk, kT))):
                wt = []
                for hc in range(16):
                    w1 = p1.tile([128, 256], F32R, name="wt", tag="wt", bufs=17)
                    nc.sync.dma_start(out=w1[:], in_=wsrc[hc * 128:(hc + 1) * 128, :])
                    wt.append(w1)
                for f in range(2):
                    ps = p1ps.tile([128, 512], F32, name="psq", tag="psq", bufs=2)
                    for hc in range(16):
                        nc.tensor.matmul(ps[:], _r(wt[hc][:, f * 128:(f + 1) * 128]),
                                         _r(hst[hc][:]), start=(hc == 0), stop=(hc == 15))
                    nc.scalar.copy(dstT[:, f * T + t0:f * T + t0 + 512], ps[:])
            # routing logits: 3-term bf16 split-GEMM (exact products, fp32 accum)
            hih, hil = [], []
            for hc in (() if "p1r" in SKIP else range(16)):
                h1 = p1.tile([128, 512], BF16, name="hih", tag="hih", bufs=17)
                nc.sync.dma_start(out=h1[:], in_=hsh[hc * 128:(hc + 1) * 128, t0:t0 + 512])
                hih.append(h1)
                h2 = p1.tile([128, 512], BF16, name="hil", tag="hil", bufs=17)
                nc.sync.dma_start(out=h2[:], in_=hsl[hc * 128:(hc + 1) * 128, t0:t0 + 512])
                hil.append(h2)
            for tl in (() if "p1r" in SKIP else range(4)):
                tt = tb * 4 + tl
                psr = p1ps.tile([128, 4], F32, name="psr", tag="psr", bufs=2)
                n_mm = 0
                for aa, bb in ((hih, wfh_sb), (hih, wfl_sb), (hil, wfh_sb)):
                    for hc in range(16):
                        nc.tensor.matmul(psr[:],
                                         aa[hc][:, tl * 128:(tl + 1) * 128],
                                         bb[:, hc * 4:(hc + 1) * 4],
                                         start=(n_mm == 0), stop=(n_mm == 47))
                        n_mm += 1
                nc.scalar.copy(logit_sb[:, tt * 4:(tt + 1) * 4], psr[:])
            for wsrc, ddst in (() if "p1vg" in SKIP else ((wv, v_dram), (wg, g_dram))):
                wt = []
                for hc in range(16):
                    w1 = p1.tile([128, 512], F32R, name="wt", tag="wt", bufs=17)
                    nc.sync.dma_start(out=w1[:], in_=wsrc[hc * 128:(hc + 1) * 128, :])
                    wt.append(w1)
                for tt4 in range(4):
                    ps = p1ps.tile([128, 512], F32, name="psv", tag="psv", bufs=2)
                    for hc in range(16):
                        nc.tensor.matmul(ps[:], _r(hst[hc][:, tt4 * 128:(tt4 + 1) * 128]),
                                         _r(wt[hc][:]), start=(hc == 0), stop=(hc == 15))
                    st = p1.tile([128, 512], F32R if ddst is v_dram else F32, name="vgst", tag="vgst", bufs=4)
                    nc.scalar.copy(st[:], ps[:])
                    nc.sync.dma_start(out=ddst[t0 + tt4 * 128:t0 + tt4 * 128 + 128, :], in_=st[:])

    # ---------------- phase 2: routing ----------------
    nc.vector.memset(rw_all[:], 0.25)
    with tc.tile_pool(name="p2", bufs=4) as p2:
        for tt in (() if "p2" in SKIP else range(16)):
            lg = logit_sb[:, tt * 4:(tt + 1) * 4]
            s = p2.tile([128, 4], F32, name="s")
            nc.scalar.activation(s[:], lg, ACTF.Exp)
            m1 = p2.tile([128, 1], F32, name="m1")
            nc.vector.tensor_reduce(m1[:], lg, axis=AX.X, op=ALU.max)
            eq = p2.tile([128, 4], F32, name="eq")
            nc.vector.tensor_scalar(eq[:], lg, m1[:], None, ALU.is_ge)
            sm = p2.tile([128, 4], F32, name="sm")
            nc.vector.scalar_tensor_tensor(sm[:], eq[:], -1e30, lg, ALU.mult, ALU.add)
            m2 = p2.tile([128, 1], F32, name="m2")
            nc.vector.tensor_reduce(m2[:], sm[:], axis=AX.X, op=ALU.max)
            sel = p2.tile([128, 4], F32, name="sel")
            nc.vector.tensor_scalar(sel[:], lg, m2[:], None, ALU.is_ge)
            w4 = p2.tile([128, 4], F32, name="w4")
            nc.vector.tensor_tensor(w4[:], s[:], sel[:], ALU.mult)
            den = p2.tile([128, 1], F32, name="den")
            nc.vector.tensor_reduce(den[:], w4[:], axis=AX.X, op=ALU.add)
            dinv = p2.tile([128, 1], F32, name="dinv")
            nc.vector.reciprocal(dinv[:], den[:])
            nc.vector.tensor_scalar(rw_all[:, tt * R + 2:tt * R + 6], w4[:], dinv[:], 0.5,
                                    ALU.mult, ALU.mult)
            nc.vector.tensor_scalar(biasN[:, tt * NE:(tt + 1) * NE], sel[:], 30.0, -30.0,
                                    ALU.mult, ALU.add)
            nc.vector.tensor_scalar(invm[:, tt * NE:(tt + 1) * NE], sel[:], -1.0, 1.0,
                                    ALU.mult, ALU.add)

    # ---------------- phase 3: expert attention ----------------
    pers3 = ctx.enter_context(tc.tile_pool(name="pers3", bufs=1))
    o_acc = pers3.tile([128, 16 * DV], F32, name="o_acc")  # per t-tile block
    with tc.tile_pool(name="p3", bufs=1) as p3, \
         tc.tile_pool(name="p3ps", bufs=1, space="PSUM") as p3ps:
        # masked-key counts per (b, routed expert): cnt[b][:, e] = #inactive keys
        cnt_sb = pers3.tile([128, 2 * NE], F32, name="cnt_sb")
        for b in range(NB):
            pscnt = p3ps.tile([128, 4], F32, name="pscnt", tag="pscnt", bufs=1)
            for kt in range(8):
                ktt = b * 8 + kt
                nc.tensor.matmul(pscnt[:], ones128[:],
                                 invm[:, ktt * NE:(ktt + 1) * NE],
                                 start=(kt == 0), stop=(kt == 7))
            nc.scalar.copy(cnt_sb[:, b * NE:(b + 1) * NE], pscnt[:])
        for r in range(R):
            # expansions qeT[r], keT[r]: [256 e, 2048 t] as 2 chunk tiles
            qeT, keT = [], []
            for wsb, lst, nm in (() if "exp" in SKIP else ((wqe_sb, qeT, "qeTt"), (wke_sb, keT, "keTt"))):
                for dco in range(2):
                    et = p3.tile([128, T], F32R, name=nm, tag=nm, bufs=3)
                    lst.append(et)
                    for nb4 in range(4):
                        ps = p3ps.tile([128, 512], F32, name="psqe", tag="psqe", bufs=2)
                        for dci in range(2):
                            nc.tensor.matmul(
                                ps[:],
                                _r(wsb[:, dci * 1536 + r * 256 + dco * 128:
                                       dci * 1536 + r * 256 + dco * 128 + 128]),
                                _r(qT[:, dci * T + nb4 * 512:dci * T + nb4 * 512 + 512]
                                   if nm == "qeTt" else
                                   kT[:, dci * T + nb4 * 512:dci * T + nb4 * 512 + 512]),
                                start=(dci == 0), stop=(dci == 1))
                        nc.scalar.copy(et[:, nb4 * 512:nb4 * 512 + 512], ps[:])
            for b in (() if "att" in SKIP else range(NB)):
                boff = b * TB
                vks = []
                for kt in range(8):
                    vt = p3.tile([128, DV], F32R, name="vks", tag="vks", bufs=10)
                    nc.sync.dma_start(out=vt[:], in_=v_dram[boff + kt * 128:boff + kt * 128 + 128, :])
                    vks.append(vt)
                for half in range(2):
                    qoff = boff + half * 512
                    expS = []
                    for kt in range(8):
                        ktt = b * 8 + kt
                        pss = p3ps.tile([128, 512], F32, name="pss", tag="pss", bufs=2)
                        for dc in range(2):
                            nc.tensor.matmul(
                                pss[:],
                                _r(keT[dc][:, boff + kt * 128:boff + kt * 128 + 128]),
                                _r(qeT[dc][:, qoff:qoff + 512]),
                                start=(dc == 0), stop=(dc == 1))
                        es = p3.tile([128, 512], F32R, name="expS", tag="expS", bufs=10)
                        if r >= 2:
                            nc.scalar.activation(
                                es[:], pss[:], ACTF.Exp, scale=SCALE,
                                bias=biasN[:, ktt * NE + (r - 2):ktt * NE + (r - 2) + 1])
                        else:
                            nc.scalar.activation(es[:], pss[:], ACTF.Exp, scale=SCALE)
                        expS.append(es)
                    psden = p3ps.tile([128, 8], F32, name="psden", tag="psden", bufs=1)
                    for j in range(4):
                        for kt in range(8):
                            nc.tensor.matmul(psden[:, 2 * j:2 * j + 2],
                                             expS[kt][:, j * 128:j * 128 + 128],
                                             ones2[:],
                                             start=(kt == 0), stop=(kt == 7))
                    for j in range(4):
                        pso = p3ps.tile([128, 512], F32, name="pso", tag="pso", bufs=2)
                        for kt in range(8):
                            nc.tensor.matmul(pso[:],
                                             _r(expS[kt][:, j * 128:j * 128 + 128]),
                                             _r(vks[kt][:]),
                                             start=(kt == 0), stop=(kt == 7))
                        tt = b * 8 + half * 4 + j
                        dinv = p3.tile([128, 1], F32, name="adinv", tag="adinv", bufs=4)
                        if r >= 2:
                            dtot = p3.tile([128, 1], F32, name="dtot", tag="dtot", bufs=4)
                            nc.vector.tensor_tensor(
                                dtot[:], psden[:, 2 * j:2 * j + 1],
                                cnt_sb[:, b * NE + (r - 2):b * NE + (r - 2) + 1], ALU.add)
                            nc.vector.reciprocal(dinv[:], dtot[:])
                        else:
                            nc.vector.reciprocal(dinv[:], psden[:, 2 * j:2 * j + 1])
                        cmul = p3.tile([128, 1], F32, name="cmul", tag="cmul", bufs=4)
                        nc.vector.tensor_tensor(cmul[:], dinv[:],
                                                rw_all[:, tt * R + r:tt * R + r + 1], ALU.mult)
                        if r == 0:
                            nc.vector.tensor_scalar(o_acc[:, tt * DV:(tt + 1) * DV],
                                                    pso[:], cmul[:], None, ALU.mult)
                        else:
                            tmp = p3.tile([128, DV], F32, name="otmp", tag="otmp", bufs=3)
                            nc.vector.tensor_scalar(tmp[:], pso[:], cmul[:], None, ALU.mult)
                            nc.vector.tensor_tensor(o_acc[:, tt * DV:(tt + 1) * DV],
                                                    o_acc[:, tt * DV:(tt + 1) * DV],
                                                    tmp[:], ALU.add)

    # ---------------- phase 4: gate, transpose, output projection ----------------
    with tc.tile_pool(name="p4", bufs=1) as p4, \
         tc.tile_pool(name="p4ps", bufs=1, space="PSUM") as p4ps:
        if "p4" in SKIP:
            return
        wo_sb = [p4.tile([128, HID], F32R, name=f"wo_sb{i}", tag=f"wo_sb{i}") for i in range(4)]
        for i in range(4):
            nc.sync.dma_start(out=wo_sb[i][:], in_=wo[i * 128:(i + 1) * 128, :])
        Xt = [p4.tile([128, T], F32R, name=f"xt{i}", tag=f"xt{i}") for i in range(4)]
        for tt in range(16):
            gsb = p4.tile([128, DV], F32, name="gsb", tag="gsb", bufs=3)
            nc.sync.dma_start(out=gsb[:], in_=g_dram[tt * 128:(tt + 1) * 128, :])
            sg = p4.tile([128, DV], F32, name="sg", tag="sg", bufs=3)
            nc.scalar.activation(sg[:], gsb[:], ACTF.Sigmoid)
            nc.vector.tensor_tensor(sg[:], sg[:], gsb[:], ALU.mult)  # silu(g)
            xres = p4.tile([128, DV], F32, name="xres", tag="xres", bufs=3)
            nc.vector.tensor_tensor(xres[:], o_acc[:, tt * DV:(tt + 1) * DV], sg[:], ALU.mult)
            for dvc in range(4):
                pst = p4ps.tile([128, 128], F32, name="pst", tag="pst", bufs=2)
                nc.tensor.transpose(pst[:], xres[:, dvc * 128:(dvc + 1) * 128], ident[:])
                nc.scalar.copy(Xt[dvc][:, tt * 128:(tt + 1) * 128], pst[:])
        for tt in range(16):
            for hb in range(4):
                psf = p4ps.tile([128, 512], F32, name="psf", tag="psf", bufs=2)
                for dvc in range(4):
                    nc.tensor.matmul(psf[:], _r(Xt[dvc][:, tt * 128:(tt + 1) * 128]),
                                     _r(wo_sb[dvc][:, hb * 512:(hb + 1) * 512]),
                                     start=(dvc == 0), stop=(dvc == 3))
                ost = p4.tile([128, 512], F32, name="ost", tag="ost", bufs=4)
                nc.scalar.copy(ost[:], psf[:])
                nc.sync.dma_start(out=out[tt * 128:(tt + 1) * 128, hb * 512:(hb + 1) * 512],
                                  in_=ost[:])


_PROGRAM = None


def build_program():
    global _PROGRAM
    if _PROGRAM is not None:
        return _PROGRAM
    from contextlib import ExitStack
    nc = bacc.Bacc("TRN2", target_bir_lowering=False, debug=False, num_devices=8)
    names = [("hsT", [HID, T], F32R), ("wq", [HID, D], F32R), ("wk", [HID, D], F32R),
             ("wv", [HID, DV], F32R), ("wg", [HID, DV], F32R), ("wqe", [D, D * R], F32R),
             ("wke", [D, D * R], F32R), ("hsh", [HID, T], BF16), ("hsl", [HID, T], BF16),
             ("wfh", [HID, NE], BF16), ("wfl", [HID, NE], BF16), ("wo", [DV, HID], F32R)]
    io = [nc.dram_tensor(n, s, dt, kind="ExternalInput").ap() for n, s, dt in names]
    io.append(nc.dram_tensor("out", [T, HID], F32, kind="ExternalOutput").ap())
    with tile.TileContext(nc) as tc:
        from contextlib import ExitStack as ES
        with ES() as ctx:
            _body(ctx, nc, tc, io)
    nc.compile()
    _PROGRAM = nc
    return nc


def make_in_maps(hidden_states, Wq, Wk, Wv, Wq_exp, Wk_exp, Wgate, Wg, Wo):
    import ml_dtypes
    bf = ml_dtypes.bfloat16
    hs2 = np.asarray(hidden_states, np.float32).reshape(T, HID)
    hsT = np.ascontiguousarray(hs2.T)
    hsh = np.ascontiguousarray(hsT.astype(bf))
    hsl = np.ascontiguousarray((hsT.astype(np.float64) - hsh.astype(np.float64)).astype(bf))
    wfus = []
    for c in range(8):
        wfu = (np.asarray(Wq, np.float64)[:, c * D:(c + 1) * D]
               @ np.asarray(Wgate, np.float64))
        wfh = wfu.astype(bf)
        wfl = (wfu - wfh.astype(np.float64)).astype(bf)
        wfus.append((np.ascontiguousarray(wfh), np.ascontiguousarray(wfl)))
    in_maps = []
    for c in range(8):
        in_maps.append({
            "hsT": hsT,
            "wq": np.ascontiguousarray(np.asarray(Wq, np.float32)[:, c * D:(c + 1) * D]),
            "wk": np.ascontiguousarray(np.asarray(Wk, np.float32)[:, c * D:(c + 1) * D]),
            "wv": np.ascontiguousarray(np.asarray(Wv, np.float32)[:, c * DV:(c + 1) * DV]),
            "wg": np.ascontiguousarray(np.asarray(Wg, np.float32)[:, c * DV:(c + 1) * DV]),
            "wqe": np.ascontiguousarray(np.asarray(Wq_exp, np.float32)[c]),
            "wke": np.ascontiguousarray(np.asarray(Wk_exp, np.float32)[c]),
            "hsh": hsh, "hsl": hsl,
            "wfh": wfus[c][0], "wfl": wfus[c][1],
            "wo": np.ascontiguousarray(np.asarray(Wo, np.float32)[c * DV:(c + 1) * DV, :]),
        })
    return in_maps


def kernel(hidden_states, Wq, Wk, Wv, Wq_exp, Wk_exp, Wgate, Wg, Wo):
    nc = build_program()
    in_maps = make_in_maps(hidden_states, Wq, Wk, Wv, Wq_exp, Wk_exp, Wgate, Wg, Wo)
    res = run_bass_kernel_spmd(nc, in_maps, list(range(8))).results
    out = np.zeros((T, HID), np.float32)
    for c in range(8):
        out += res[c]["out"]
    return out.reshape(2, 1024, HID).astype(np.float32)



# revision 8
# speedup vs baseline: 1.0317x; 1.0317x over previous
"""Trainium2 Bass kernel for MockMobGatedDeltaNetMoE (v2).

Sharding: head-parallel over H=8 heads, one head per NeuronCore.
Each core computes its head's full contribution; the host sums the 8
partial output projections.

v2 reformulation (exact-equivalent of the reference):
 - Score fusion: S_r = q @ (Wq_exp_r @ Wk_exp_r^T) @ k^T. M_r fused on host
   (fp64), so no ke expansion on device.
 - Key masking: k is masked per routed expert in [token, d] layout
   (per-partition scalars), then PE-transposed to [d, token] for the score
   matmuls. Masked keys give S=0 -> exp(0)=1, which matches the reference
   denominator exactly (reference has ke=0 there).
 - Scores are computed TRANSPOSED: [query-part, key-free]. exp runs with
   accum_out, yielding the softmax denominator for free.
 - Expert combine in PSUM: P~^T[k,q] = sum_r expS_r^T[q,k]^T @ diag(c_r)
   where c_{r,q} = rw_{r,q} / den_{r,q}. One diagonal matmul per
   (expert, k-tile) both transposes and scales -- the per-expert AV matmuls
   collapse into ONE attention @ V matmul per batch.
 - Masked keys contribute 1*v[k] spuriously to the numerator; corrected by
   one rank-4 matmul per query tile: avp += C^T[r,q] @ (-sum_masked v).
 - Router: exact 3-term bf16 split GEMM (unchanged from v1); fp32r logits
   flip one top-2 decision (costs 6e-3 rel err), so we keep the exact path.
All matmuls run as float32r (full fp32 data, fast PE mode).
"""

import numpy as np

import concourse.bass as bass
import concourse.bacc as bacc
import concourse.tile as tile
from concourse import mybir
from concourse.bass_utils import run_bass_kernel_spmd

F32 = mybir.dt.float32
F32R = mybir.dt.float32r
BF16 = mybir.dt.bfloat16
ALU = mybir.AluOpType
ACTF = mybir.ActivationFunctionType
AX = mybir.AxisListType

H, D, R, NE = 8, 256, 6, 4          # heads, head_dim, experts, routed experts
HID, DV, T = 2048, 512, 2048        # hidden, head_v_dim, b*t tokens
NB = 2                              # batch
TB = T // NB                        # tokens per batch (attention window)
SCALE = 1.0 / 16.0                  # 1/sqrt(D)


def _body(ctx, nc, tc, io):
    hsT, wq, wk, wv, wg, wqm, hsh, hsl, wfh, wfl, wo, out = io

    const = ctx.enter_context(tc.tile_pool(name="const", bufs=1))
    pers = ctx.enter_context(tc.tile_pool(name="pers", bufs=1))

    ones_f32 = const.tile([128, 128], F32, name="ones_f32")
    nc.vector.memset(ones_f32[:], 1.0)
    from concourse.masks import make_identity
    ident = const.tile([128, 128], F32, name="ident")
    make_identity(nc, ident)
    # fused routing weight (Wq_head @ Wgate, host-fp64) split hi/lo bf16
    wfh_sb = const.tile([128, 64], BF16, name="wfh_sb")
    wfl_sb = const.tile([128, 64], BF16, name="wfl_sb")
    for hc in range(16):
        nc.sync.dma_start(out=wfh_sb[:, hc * 4:(hc + 1) * 4],
                          in_=wfh[hc * 128:(hc + 1) * 128, :])
        nc.sync.dma_start(out=wfl_sb[:, hc * 4:(hc + 1) * 4],
                          in_=wfl[hc * 128:(hc + 1) * 128, :])
    logit_sb = pers.tile([128, 64], F32, name="logit_sb")

    # persistent tensors
    qT = pers.tile([128, 2 * T], F32R, name="qT")        # [d-chunk, token]
    k_sb = pers.tile([128, 16 * D], F32, name="k_sb")    # [token-tile, d]
    v_sb = pers.tile([128, 16 * DV], F32R, name="v_sb")  # [token-tile, dv]
    wqm_sb = pers.tile([128, 2 * 1536], F32R, name="wqm_sb")  # M_r fused
    rw_all = pers.tile([128, 16 * R], F32, name="rw_all")
    msel = pers.tile([128, 16 * NE], F32, name="msel")   # top-2 mask (1/0)
    nsel = pers.tile([128, 16 * NE], F32R, name="nsel")  # sel - 1 (0/-1)
    for dc in range(2):
        nc.sync.dma_start(out=wqm_sb[:, dc * 1536:(dc + 1) * 1536],
                          in_=wqm[dc * 128:(dc + 1) * 128, :])

    dram = ctx.enter_context(tc.tile_pool(name="dram", bufs=1, space="DRAM"))
    g_dram = dram.tile([T, DV], F32, name="g_dram")

    # ---------------- phase 1: projections ----------------
    with tc.tile_pool(name="p1", bufs=1) as p1, \
         tc.tile_pool(name="p1ps", bufs=1, space="PSUM") as p1ps:
        for tb in range(4):  # token blocks of 512
            t0 = tb * 512
            hst = []
            for hc in range(16):
                ht = p1.tile([128, 512], F32R, name="hst", tag="hst", bufs=17)
                nc.sync.dma_start(out=ht[:], in_=hsT[hc * 128:(hc + 1) * 128, t0:t0 + 512])
                hst.append(ht)
            # q projection -> qT [d-chunk, token]
            wt = []
            for hc in range(16):
                w1 = p1.tile([128, 256], F32R, name="wtq", tag="wtq", bufs=17)
                nc.sync.dma_start(out=w1[:], in_=wq[hc * 128:(hc + 1) * 128, :])
                wt.append(w1)
            for f in range(2):
                ps = p1ps.tile([128, 512], F32, name="psq", tag="psq", bufs=2)
                for hc in range(16):
                    nc.tensor.matmul(ps[:], wt[hc][:, f * 128:(f + 1) * 128],
                                     hst[hc][:], start=(hc == 0), stop=(hc == 15))
                nc.scalar.copy(qT[:, f * T + t0:f * T + t0 + 512], ps[:])
            # k projection -> k_sb [token, d]
            wtk = []
            for hc in range(16):
                w1 = p1.tile([128, 256], F32R, name="wtk", tag="wtq", bufs=17)
                nc.sync.dma_start(out=w1[:], in_=wk[hc * 128:(hc + 1) * 128, :])
                wtk.append(w1)
            for tt4 in range(4):
                ps = p1ps.tile([128, 256], F32, name="psk", tag="psk", bufs=2)
                for hc in range(16):
                    nc.tensor.matmul(ps[:], hst[hc][:, tt4 * 128:(tt4 + 1) * 128],
                                     wtk[hc][:], start=(hc == 0), stop=(hc == 15))
                tt = tb * 4 + tt4
                nc.scalar.copy(k_sb[:, tt * D:(tt + 1) * D], ps[:])
            # routing logits: 3-term bf16 split-GEMM (exact products, fp32 accum)
            hih, hil = [], []
            for hc in range(16):
                h1 = p1.tile([128, 512], BF16, name="hih", tag="hih", bufs=17)
                nc.sync.dma_start(out=h1[:], in_=hsh[hc * 128:(hc + 1) * 128, t0:t0 + 512])
                hih.append(h1)
                h2 = p1.tile([128, 512], BF16, name="hil", tag="hil", bufs=17)
                nc.sync.dma_start(out=h2[:], in_=hsl[hc * 128:(hc + 1) * 128, t0:t0 + 512])
                hil.append(h2)
            for tl in range(4):
                tt = tb * 4 + tl
                psr = p1ps.tile([128, 4], F32, name="psr", tag="psr", bufs=2)
                n_mm = 0
                for aa, bb in ((hih, wfh_sb), (hih, wfl_sb), (hil, wfh_sb)):
                    for hc in range(16):
                        nc.tensor.matmul(psr[:],
                                         aa[hc][:, tl * 128:(tl + 1) * 128],
                                         bb[:, hc * 4:(hc + 1) * 4],
                                         start=(n_mm == 0), stop=(n_mm == 47))
                        n_mm += 1
                nc.scalar.copy(logit_sb[:, tt * 4:(tt + 1) * 4], psr[:])
            # v -> v_sb, g -> g_dram
            for wsrc, vdst in ((wv, "v"), (wg, "g")):
                wt2 = []
                for hc in range(16):
                    w1 = p1.tile([128, 512], F32R, name="wtv", tag="wtv", bufs=17)
                    nc.sync.dma_start(out=w1[:], in_=wsrc[hc * 128:(hc + 1) * 128, :])
                    wt2.append(w1)
                for tt4 in range(4):
                    ps = p1ps.tile([128, 512], F32, name="psv", tag="psv", bufs=2)
                    for hc in range(16):
                        nc.tensor.matmul(ps[:], hst[hc][:, tt4 * 128:(tt4 + 1) * 128],
                                         wt2[hc][:], start=(hc == 0), stop=(hc == 15))
                    tt = tb * 4 + tt4
                    if vdst == "v":
                        nc.scalar.copy(v_sb[:, tt * DV:(tt + 1) * DV], ps[:])
                    else:
                        st = p1.tile([128, 512], F32, name="gst", tag="gst", bufs=4)
                        nc.scalar.copy(st[:], ps[:])
                        nc.sync.dma_start(out=g_dram[tt * 128:(tt + 1) * 128, :], in_=st[:])

    # ---------------- phase 2: routing ----------------
    nc.vector.memset(rw_all[:], 0.25)
    with tc.tile_pool(name="p2", bufs=4) as p2:
        for tt in range(16):
            lg = logit_sb[:, tt * 4:(tt + 1) * 4]
            s = p2.tile([128, 4], F32, name="s")
            nc.scalar.activation(s[:], lg, ACTF.Exp)
            m1 = p2.tile([128, 1], F32, name="m1")
            nc.vector.tensor_reduce(m1[:], lg, axis=AX.X, op=ALU.max)
            eq = p2.tile([128, 4], F32, name="eq")
            nc.vector.tensor_scalar(eq[:], lg, m1[:], None, ALU.is_ge)
            sm = p2.tile([128, 4], F32, name="sm")
            nc.vector.scalar_tensor_tensor(sm[:], eq[:], -1e30, lg, ALU.mult, ALU.add)
            m2 = p2.tile([128, 1], F32, name="m2")
            nc.vector.tensor_reduce(m2[:], sm[:], axis=AX.X, op=ALU.max)
            sel = p2.tile([128, 4], F32, name="sel")
            nc.vector.tensor_scalar(sel[:], lg, m2[:], None, ALU.is_ge)
            w4 = p2.tile([128, 4], F32, name="w4")
            nc.vector.tensor_tensor(w4[:], s[:], sel[:], ALU.mult)
            den = p2.tile([128, 1], F32, name="den")
            nc.vector.tensor_reduce(den[:], w4[:], axis=AX.X, op=ALU.add)
            dinv = p2.tile([128, 1], F32, name="dinv")
            nc.vector.reciprocal(dinv[:], den[:])
            nc.vector.tensor_scalar(rw_all[:, tt * R + 2:tt * R + 6], w4[:], dinv[:], 0.5,
                                    ALU.mult, ALU.mult)
            nc.scalar.copy(msel[:, tt * NE:(tt + 1) * NE], sel[:])
            nc.vector.tensor_scalar(nsel[:, tt * NE:(tt + 1) * NE], sel[:], 1.0, -1.0,
                                    ALU.mult, ALU.add)

    # ---------------- phase 3: expert attention (combined) ----------------
    pers3 = ctx.enter_context(tc.tile_pool(name="pers3", bufs=1))
    o_acc = pers3.tile([128, 16 * DV], F32, name="o_acc")
    with tc.tile_pool(name="p3", bufs=1) as p3, \
         tc.tile_pool(name="p3ps", bufs=1, space="PSUM") as p3ps:
        for b in range(NB):
            # --- kTm: masked-transposed keys, 5 sets (shared + 4 routed) ---
            ktm = []  # [set][dc] -> [128, TB]
            for rs in range(5):
                pair = [p3.tile([128, TB], F32R, name=f"ktm", tag=f"ktm{rs}{dc}", bufs=1)
                        for dc in range(2)]
                ktm.append(pair)
                for kt in range(8):
                    ktt = b * 8 + kt
                    if rs == 0:
                        src = k_sb[:, ktt * D:(ktt + 1) * D]
                    else:
                        km = p3.tile([128, D], F32, name="km", tag="km", bufs=2)
                        nc.vector.tensor_scalar(
                            km[:], k_sb[:, ktt * D:(ktt + 1) * D],
                            msel[:, ktt * NE + (rs - 1):ktt * NE + rs], None, ALU.mult)
                        src = km[:]
                    for dc in range(2):
                        pst = p3ps.tile([128, 128], F32, name="pst", tag="ps_misc", bufs=2)
                        nc.tensor.transpose(pst[:], src[:, dc * 128:(dc + 1) * 128], ident[:])
                        nc.vector.tensor_copy(pair[dc][:, kt * 128:(kt + 1) * 128], pst[:])
            # --- nspur_b[r', :] = -sum_{masked k} v[k, :]  (rank-4) ---
            psn = p3ps.tile([128, DV], F32, name="psn", tag="ps_misc", bufs=2)
            for kt in range(8):
                ktt = b * 8 + kt
                nc.tensor.matmul(psn[0:NE, :], nsel[:, ktt * NE:(ktt + 1) * NE],
                                 v_sb[:, ktt * DV:(ktt + 1) * DV],
                                 start=(kt == 0), stop=(kt == 7))
            nspur = p3.tile([NE, DV], F32R, name="nspur", tag="nspur", bufs=1)
            nc.scalar.copy(nspur[:], psn[0:NE, :])
            for qh in range(2):  # query halves of 512
                # qmT for all r: [r][d2c] -> [128, 512]
                qmT = []
                for r in range(R):
                    pair = []
                    for d2c in range(2):
                        psq = p3ps.tile([128, 512], F32, name="psqm", tag="ps_misc", bufs=2)
                        for dc in range(2):
                            nc.tensor.matmul(
                                psq[:],
                                wqm_sb[:, dc * 1536 + r * 256 + d2c * 128:
                                       dc * 1536 + r * 256 + d2c * 128 + 128],
                                qT[:, dc * T + b * TB + qh * 512:
                                   dc * T + b * TB + qh * 512 + 512],
                                start=(dc == 0), stop=(dc == 1))
                        qm = p3.tile([128, 512], F32R, name="qmT", tag=f"qmT{r}{d2c}", bufs=1)
                        nc.vector.tensor_copy(qm[:], psq[:])
                        pair.append(qm)
                    qmT.append(pair)
                for qt in range(4):  # query tiles of 128
                    tt = b * 8 + qh * 4 + qt
                    q0 = qt * 128
                    ptps = p3ps.tile([128, 1024], F32, name="ptps", tag="ptps", bufs=1)
                    csb = p3.tile([128, NE], F32, name="csb", tag="csb", bufs=2)
                    for r in range(R):
                        krs = 0 if r < 2 else r - 1
                        # scores S^T [q, k] in two 512 chunks
                        es_pair = []
                        dsum = []
                        for kc in range(2):
                            sps = p3ps.tile([128, 512], F32, name="sps", tag="sps", bufs=2)
                            for d2c in range(2):
                                nc.tensor.matmul(
                                    sps[:], qmT[r][d2c][:, q0:q0 + 128],
                                    ktm[krs][d2c][:, kc * 512:(kc + 1) * 512],
                                    start=(d2c == 0), stop=(d2c == 1))
                            es = p3.tile([128, 512], F32R, name="es", tag="es", bufs=4)
                            dn = p3.tile([128, 1], F32, name="dn", tag="dn", bufs=4)
                            nc.scalar.activation(es[:], sps[:], ACTF.Exp, scale=SCALE,
                                                 accum_out=dn[:])
                            es_pair.append(es)
                            dsum.append(dn)
                        dtot = p3.tile([128, 1], F32, name="dtot", tag="dtot", bufs=2)
                        nc.vector.tensor_tensor(dtot[:], dsum[0][:], dsum[1][:], ALU.add)
                        dinv = p3.tile([128, 1], F32, name="adinv", tag="adinv", bufs=2)
                        nc.vector.reciprocal(dinv[:], dtot[:])
                        cmul = p3.tile([128, 1], F32, name="cmul", tag="cmul", bufs=2)
                        nc.vector.tensor_tensor(cmul[:], dinv[:],
                                                rw_all[:, tt * R + r:tt * R + r + 1],
                                                ALU.mult)
                        if r >= 2:
                            nc.scalar.copy(csb[:, r - 2:r - 1], cmul[:])
                        dcd = p3.tile([128, 128], F32R, name="dcd", tag="dcd", bufs=2)
                        nc.vector.tensor_scalar(dcd[:], ident[:], cmul[:], None, ALU.mult)
                        # combine: P~T[k-tile, q-tile] += expS^T[:,kt]^T @ diag(c)
                        # start only on the first matmul touching each PSUM
                        # bank -- start=True clears has_written for the WHOLE
                        # bank, which would wipe sibling k-tile slices.
                        for kt in range(8):
                            nc.tensor.matmul(
                                ptps[:, kt * 128:(kt + 1) * 128],
                                es_pair[kt // 4][:, (kt % 4) * 128:(kt % 4) * 128 + 128],
                                dcd[:], start=(r == 0 and kt % 4 == 0),
                                stop=(r == R - 1))
                    pts = p3.tile([128, 1024], F32R, name="pts", tag="pts", bufs=2)
                    nc.scalar.copy(pts[:], ptps[:])
                    # CT: [4, 128] = csb^T
                    psc = p3ps.tile([128, 128], F32, name="psc", tag="ps_ct", bufs=1)
                    nc.tensor.matmul(psc[0:NE, :], csb[:], ident[:], start=True, stop=True)
                    ctb = p3.tile([NE, 128], F32R, name="ctb", tag="ctb", bufs=2)
                    nc.scalar.copy(ctb[:], psc[0:NE, :])
                    # AV + spur correction
                    avp = p3ps.tile([128, DV], F32, name="avp", tag="avp", bufs=1)
                    for kt in range(8):
                        ktt = b * 8 + kt
                        nc.tensor.matmul(avp[:], pts[:, kt * 128:(kt + 1) * 128],
                                         v_sb[:, ktt * DV:(ktt + 1) * DV],
                                         start=(kt == 0), stop=False)
                    nc.tensor.matmul(avp[:], ctb[:], nspur[:], start=False, stop=True)
                    nc.vector.tensor_copy(o_acc[:, tt * DV:(tt + 1) * DV], avp[:])

    # ---------------- phase 4: gate, transpose, output projection ----------------
    with tc.tile_pool(name="p4", bufs=1) as p4, \
         tc.tile_pool(name="p4ps", bufs=1, space="PSUM") as p4ps:
        wo_sb = [p4.tile([128, HID], F32R, name=f"wo_sb{i}", tag=f"wo_sb{i}") for i in range(4)]
        for i in range(4):
            nc.sync.dma_start(out=wo_sb[i][:], in_=wo[i * 128:(i + 1) * 128, :])
        Xt = [p4.tile([128, T], F32R, name=f"xt{i}", tag=f"xt{i}") for i in range(4)]
        for tt in range(16):
            gsb = p4.tile([128, DV], F32, name="gsb", tag="gsb", bufs=3)
            nc.sync.dma_start(out=gsb[:], in_=g_dram[tt * 128:(tt + 1) * 128, :])
            sg = p4.tile([128, DV], F32, name="sg", tag="sg", bufs=3)
            nc.scalar.activation(sg[:], gsb[:], ACTF.Sigmoid)
            nc.vector.tensor_tensor(sg[:], sg[:], gsb[:], ALU.mult)  # silu(g)
            xres = p4.tile([128, DV], F32, name="xres", tag="xres", bufs=3)
            nc.vector.tensor_tensor(xres[:], o_acc[:, tt * DV:(tt + 1) * DV], sg[:], ALU.mult)
            for dvc in range(4):
                pst = p4ps.tile([128, 128], F32, name="pst4", tag="pst4", bufs=2)
                nc.tensor.transpose(pst[:], xres[:, dvc * 128:(dvc + 1) * 128], ident[:])
                nc.scalar.copy(Xt[dvc][:, tt * 128:(tt + 1) * 128], pst[:])
        for tt in range(16):
            for hb in range(4):
                psf = p4ps.tile([128, 512], F32, name="psf", tag="psf", bufs=2)
                for dvc in range(4):
                    nc.tensor.matmul(psf[:], Xt[dvc][:, tt * 128:(tt + 1) * 128],
                                     wo_sb[dvc][:, hb * 512:(hb + 1) * 512],
                                     start=(dvc == 0), stop=(dvc == 3))
                ost = p4.tile([128, 512], F32, name="ost", tag="ost", bufs=4)
                nc.scalar.copy(ost[:], psf[:])
                nc.sync.dma_start(out=out[tt * 128:(tt + 1) * 128, hb * 512:(hb + 1) * 512],
                                  in_=ost[:])


_PROGRAM = None


def build_program():
    global _PROGRAM
    if _PROGRAM is not None:
        return _PROGRAM
    nc = bacc.Bacc("TRN2", target_bir_lowering=False, debug=False, num_devices=8)
    names = [("hsT", [HID, T], F32R), ("wq", [HID, D], F32R), ("wk", [HID, D], F32R),
             ("wv", [HID, DV], F32R), ("wg", [HID, DV], F32R),
             ("wqm", [D, D * R], F32R),
             ("hsh", [HID, T], BF16), ("hsl", [HID, T], BF16),
             ("wfh", [HID, NE], BF16), ("wfl", [HID, NE], BF16), ("wo", [DV, HID], F32R)]
    io = [nc.dram_tensor(n, s, dt, kind="ExternalInput").ap() for n, s, dt in names]
    io.append(nc.dram_tensor("out", [T, HID], F32, kind="ExternalOutput").ap())
    with tile.TileContext(nc) as tc:
        from contextlib import ExitStack as ES
        with ES() as ctx:
            _body(ctx, nc, tc, io)
    nc.compile()
    _PROGRAM = nc
    return nc


def make_in_maps(hidden_states, Wq, Wk, Wv, Wq_exp, Wk_exp, Wgate, Wg, Wo):
    import ml_dtypes
    bf = ml_dtypes.bfloat16
    hs2 = np.asarray(hidden_states, np.float32).reshape(T, HID)
    hsT = np.ascontiguousarray(hs2.T)
    hsh = np.ascontiguousarray(hsT.astype(bf))
    hsl = np.ascontiguousarray((hsT.astype(np.float64) - hsh.astype(np.float64)).astype(bf))
    Wq64 = np.asarray(Wq, np.float64)
    Wg64 = np.asarray(Wgate, np.float64)
    Wqe64 = np.asarray(Wq_exp, np.float64)
    Wke64 = np.asarray(Wk_exp, np.float64)
    in_maps = []
    for c in range(8):
        wfu = Wq64[:, c * D:(c + 1) * D] @ Wg64
        wfh = wfu.astype(bf)
        wfl = (wfu - wfh.astype(np.float64)).astype(bf)
        # M_r = Wq_exp_r @ Wk_exp_r^T  [d_orig(contract-with-k), d2] per r
        wqm = np.empty((D, D * R), np.float32)
        for r in range(R):
            m = Wqe64[c][:, r * D:(r + 1) * D] @ Wke64[c][:, r * D:(r + 1) * D].T
            # qm = q @ M_r : lhsT chunks are M_r[d_q, d2]; contraction with k
            # happens over d2 == k's d axis, so ship M_r as [d_q, d2] ... but
            # the qm matmul contracts over d_q (q's axis):
            # qmT[d2, t] = sum_dq M_r[dq, d2] * qT[dq, t]  -> lhsT = M_r.
            wqm[:, r * D:(r + 1) * D] = m.astype(np.float32)
        in_maps.append({
            "hsT": hsT,
            "wq": np.ascontiguousarray(np.asarray(Wq, np.float32)[:, c * D:(c + 1) * D]),
            "wk": np.ascontiguousarray(np.asarray(Wk, np.float32)[:, c * D:(c + 1) * D]),
            "wv": np.ascontiguousarray(np.asarray(Wv, np.float32)[:, c * DV:(c + 1) * DV]),
            "wg": np.ascontiguousarray(np.asarray(Wg, np.float32)[:, c * DV:(c + 1) * DV]),
            "wqm": wqm,
            "hsh": hsh, "hsl": hsl,
            "wfh": np.ascontiguousarray(wfh), "wfl": np.ascontiguousarray(wfl),
            "wo": np.ascontiguousarray(np.asarray(Wo, np.float32)[c * DV:(c + 1) * DV, :]),
        })
    return in_maps


def kernel(hidden_states, Wq, Wk, Wv, Wq_exp, Wk_exp, Wgate, Wg, Wo):
    nc = build_program()
    in_maps = make_in_maps(hidden_states, Wq, Wk, Wv, Wq_exp, Wk_exp, Wgate, Wg, Wo)
    res = run_bass_kernel_spmd(nc, in_maps, list(range(8))).results
    out = np.zeros((T, HID), np.float32)
    for c in range(8):
        out += res[c]["out"]
    return out.reshape(2, 1024, HID).astype(np.float32)


# revision 10
# speedup vs baseline: 1.4292x; 1.3853x over previous
"""Trainium2 Bass kernel for MockMobGatedDeltaNetMoE (v3).

Sharding: head-parallel over H=8 heads, one head per NeuronCore.
Each core computes its head's full contribution; the host sums the 8
partial output projections.

v3 = v2 reformulation + fp16 compute path + weights-loaded-once phase 1:
 - Hidden states ship ONLY as an exact bf16 hi/lo pair (router needs exact
   fp32 logits for stable top-2); fp16 hs for the projections is derived
   on-device as hi+lo (one DVE add per tile).
 - All projection/attention matmuls run in fp16 (operand rounding ~5e-4):
   2-byte dtype enables FWL fast weight load; PSUM accumulation stays fp32.
 - Score fusion: S_r = q @ (Wq_exp_r @ Wk_exp_r^T) @ k^T, M_r fused on host.
 - Key masking per routed expert in [token, d] layout, PE-transposed to
   [d, token]; masked keys give exp(0)=1 = the reference denominator.
 - Scores computed transposed [query, key]; exp emits the denominator via
   accum_out.
 - Expert combine in PSUM via diagonal matmuls (transpose + scale +
   accumulate in one instruction per (expert, key-tile)); single
   attention @ V matmul per query tile + rank-4 masked-v correction.
"""

import numpy as np

import concourse.bass as bass
import concourse.bacc as bacc
import concourse.tile as tile
from concourse import mybir
from concourse.bass_utils import run_bass_kernel_spmd

F32 = mybir.dt.float32
F16 = mybir.dt.float16
BF16 = mybir.dt.bfloat16
ALU = mybir.AluOpType
ACTF = mybir.ActivationFunctionType
AX = mybir.AxisListType

H, D, R, NE = 8, 256, 6, 4          # heads, head_dim, experts, routed experts
HID, DV, T = 2048, 512, 2048        # hidden, head_v_dim, b*t tokens
NB = 2                              # batch
TB = T // NB                        # tokens per batch (attention window)
SCALE = 1.0 / 16.0                  # 1/sqrt(D)


def _body(ctx, nc, tc, io):
    wq, wk, wv, wg, wqm, hsh, hsl, wfh, wfl, wo, out = io

    const = ctx.enter_context(tc.tile_pool(name="const", bufs=1))
    pers = ctx.enter_context(tc.tile_pool(name="pers", bufs=1))

    from concourse.masks import make_identity
    ident = const.tile([128, 128], F32, name="ident")
    make_identity(nc, ident)
    ident16 = const.tile([128, 128], F16, name="ident16")
    nc.vector.tensor_copy(ident16[:], ident[:])
    wfh_sb = const.tile([128, 64], BF16, name="wfh_sb")
    wfl_sb = const.tile([128, 64], BF16, name="wfl_sb")
    for hc in range(16):
        nc.sync.dma_start(out=wfh_sb[:, hc * 4:(hc + 1) * 4],
                          in_=wfh[hc * 128:(hc + 1) * 128, :])
        nc.sync.dma_start(out=wfl_sb[:, hc * 4:(hc + 1) * 4],
                          in_=wfl[hc * 128:(hc + 1) * 128, :])
    logit_sb = pers.tile([128, 64], F32, name="logit_sb")

    # persistent tensors (fp16)
    qT = pers.tile([128, 2 * T], F16, name="qT")         # [d-chunk, token]
    k_sb = pers.tile([128, 16 * D], F16, name="k_sb")    # [token-tile, d]
    v_sb = pers.tile([128, 16 * DV], F16, name="v_sb")   # [token-tile, dv]
    g_sb = pers.tile([128, 16 * DV], F16, name="g_sb")   # [token-tile, dv]
    wqm_sb = pers.tile([128, 2 * 1536], F16, name="wqm_sb")
    rw_all = pers.tile([128, 16 * R], F32, name="rw_all")
    msel = pers.tile([128, 16 * NE], F32, name="msel")   # top-2 mask (1/0)
    nsel = pers.tile([128, 16 * NE], F16, name="nsel")   # sel - 1 (0/-1)
    for dc in range(2):
        nc.sync.dma_start(out=wqm_sb[:, dc * 1536:(dc + 1) * 1536],
                          in_=wqm[dc * 128:(dc + 1) * 128, :])

    # ---------------- phase 1: projections (weights loaded once) ----------------
    with tc.tile_pool(name="p1w", bufs=1) as p1w, \
         tc.tile_pool(name="p1", bufs=1) as p1, \
         tc.tile_pool(name="p1ps", bufs=1, space="PSUM") as p1ps:
        wq_sb, wk_sb, wv_sb, wg_sb = [], [], [], []
        for hc in range(16):
            for lst, src, wdt, nm in ((wq_sb, wq, 256, "wqsb"), (wk_sb, wk, 256, "wksb"),
                                      (wv_sb, wv, 512, "wvsb"), (wg_sb, wg, 512, "wgsb")):
                w1 = p1w.tile([128, wdt], F16, name=nm, tag=f"{nm}{hc}")
                nc.sync.dma_start(out=w1[:], in_=src[hc * 128:(hc + 1) * 128, :])
                lst.append(w1)
        for tb in range(4):  # token blocks of 512
            t0 = tb * 512
            hih, hil, hst = [], [], []
            for hc in range(16):
                h1 = p1.tile([128, 512], BF16, name="hih", tag="hih", bufs=17)
                nc.sync.dma_start(out=h1[:], in_=hsh[hc * 128:(hc + 1) * 128, t0:t0 + 512])
                hih.append(h1)
                h2 = p1.tile([128, 512], BF16, name="hil", tag="hil", bufs=17)
                nc.sync.dma_start(out=h2[:], in_=hsl[hc * 128:(hc + 1) * 128, t0:t0 + 512])
                hil.append(h2)
                h3 = p1.tile([128, 512], F16, name="hst", tag="hst", bufs=17)
                nc.vector.tensor_tensor(h3[:], h1[:], h2[:], ALU.add)  # fp16 hs
                hst.append(h3)
            # q projection -> qT [d-chunk, token]
            for f in range(2):
                ps = p1ps.tile([128, 512], F32, name="psq", tag="psq", bufs=2)
                for hc in range(16):
                    nc.tensor.matmul(ps[:], wq_sb[hc][:, f * 128:(f + 1) * 128],
                                     hst[hc][:], start=(hc == 0), stop=(hc == 15))
                nc.scalar.copy(qT[:, f * T + t0:f * T + t0 + 512], ps[:])
            # k/v/g: one shared-stationary pass per token tile
            for tt4 in range(4):
                tt = tb * 4 + tt4
                ps = p1ps.tile([128, 1536], F32, name="pskvg", tag="pskvg", bufs=2)
                for hc in range(16):
                    st_ap = hst[hc][:, tt4 * 128:(tt4 + 1) * 128]
                    nc.tensor.matmul(ps[:, 0:256], st_ap, wk_sb[hc][:],
                                     start=(hc == 0), stop=(hc == 15))
                    nc.tensor.matmul(ps[:, 512:1024], st_ap, wv_sb[hc][:],
                                     start=(hc == 0), stop=(hc == 15))
                    nc.tensor.matmul(ps[:, 1024:1536], st_ap, wg_sb[hc][:],
                                     start=(hc == 0), stop=(hc == 15))
                nc.scalar.copy(k_sb[:, tt * D:(tt + 1) * D], ps[:, 0:256])
                nc.scalar.copy(v_sb[:, tt * DV:(tt + 1) * DV], ps[:, 512:1024])
                nc.vector.tensor_copy(g_sb[:, tt * DV:(tt + 1) * DV], ps[:, 1024:1536])
            # routing logits: 3-term bf16 split-GEMM (exact products, fp32 accum)
            for tl in range(4):
                tt = tb * 4 + tl
                psr = p1ps.tile([128, 4], F32, name="psr", tag="psq", bufs=2)
                n_mm = 0
                for aa, bb in ((hih, wfh_sb), (hih, wfl_sb), (hil, wfh_sb)):
                    for hc in range(16):
                        nc.tensor.matmul(psr[:],
                                         aa[hc][:, tl * 128:(tl + 1) * 128],
                                         bb[:, hc * 4:(hc + 1) * 4],
                                         start=(n_mm == 0), stop=(n_mm == 47))
                        n_mm += 1
                nc.scalar.copy(logit_sb[:, tt * 4:(tt + 1) * 4], psr[:])

    # ---------------- phase 2: routing ----------------
    nc.vector.memset(rw_all[:], 0.25)
    with tc.tile_pool(name="p2", bufs=4) as p2:
        for tt in range(16):
            lg = logit_sb[:, tt * 4:(tt + 1) * 4]
            s = p2.tile([128, 4], F32, name="s")
            nc.scalar.activation(s[:], lg, ACTF.Exp)
            m1 = p2.tile([128, 1], F32, name="m1")
            nc.vector.tensor_reduce(m1[:], lg, axis=AX.X, op=ALU.max)
            eq = p2.tile([128, 4], F32, name="eq")
            nc.vector.tensor_scalar(eq[:], lg, m1[:], None, ALU.is_ge)
            sm = p2.tile([128, 4], F32, name="sm")
            nc.vector.scalar_tensor_tensor(sm[:], eq[:], -1e30, lg, ALU.mult, ALU.add)
            m2 = p2.tile([128, 1], F32, name="m2")
            nc.vector.tensor_reduce(m2[:], sm[:], axis=AX.X, op=ALU.max)
            sel = p2.tile([128, 4], F32, name="sel")
            nc.vector.tensor_scalar(sel[:], lg, m2[:], None, ALU.is_ge)
            w4 = p2.tile([128, 4], F32, name="w4")
            nc.vector.tensor_tensor(w4[:], s[:], sel[:], ALU.mult)
            den = p2.tile([128, 1], F32, name="den")
            nc.vector.tensor_reduce(den[:], w4[:], axis=AX.X, op=ALU.add)
            dinv = p2.tile([128, 1], F32, name="dinv")
            nc.vector.reciprocal(dinv[:], den[:])
            nc.vector.tensor_scalar(rw_all[:, tt * R + 2:tt * R + 6], w4[:], dinv[:], 0.5,
                                    ALU.mult, ALU.mult)
            nc.scalar.copy(msel[:, tt * NE:(tt + 1) * NE], sel[:])
            nc.vector.tensor_scalar(nsel[:, tt * NE:(tt + 1) * NE], sel[:], 1.0, -1.0,
                                    ALU.mult, ALU.add)

    # ---------------- phase 3: expert attention (combined) ----------------
    pers3 = ctx.enter_context(tc.tile_pool(name="pers3", bufs=1))
    o_acc = pers3.tile([128, 16 * DV], F32, name="o_acc")
    wo_sb = [pers3.tile([128, HID], F16, name=f"wo_sb{i}") for i in range(4)]
    with tc.tile_pool(name="p3", bufs=1) as p3, \
         tc.tile_pool(name="p3ps", bufs=1, space="PSUM") as p3ps:
        for b in range(NB):
            # --- kTm: masked-transposed keys, 5 sets (shared + 4 routed) ---
            ktm = []  # [set][dc] -> [128, TB] fp16
            for rs in range(5):
                pair = [p3.tile([128, TB], F16, name="ktm", tag=f"ktm{rs}{dc}", bufs=1)
                        for dc in range(2)]
                ktm.append(pair)
                for kt in range(8):
                    ktt = b * 8 + kt
                    if rs == 0:
                        src = k_sb[:, ktt * D:(ktt + 1) * D]
                    else:
                        km = p3.tile([128, D], F16, name="km", tag="km", bufs=2)
                        nc.vector.tensor_scalar(
                            km[:], k_sb[:, ktt * D:(ktt + 1) * D],
                            msel[:, ktt * NE + (rs - 1):ktt * NE + rs], None, ALU.mult)
                        src = km[:]
                    for dc in range(2):
                        pst = p3ps.tile([128, 128], F16, name="pst", tag="ps_misc", bufs=2)
                        nc.tensor.transpose(pst[:], src[:, dc * 128:(dc + 1) * 128],
                                            ident16[:])
                        nc.vector.tensor_copy(pair[dc][:, kt * 128:(kt + 1) * 128], pst[:])
            # --- nspur_b[r', :] = -sum_{masked k} v[k, :]  (rank-4) ---
            psn = p3ps.tile([128, DV], F32, name="psn", tag="ps_misc", bufs=2)
            for kt in range(8):
                ktt = b * 8 + kt
                nc.tensor.matmul(psn[0:NE, :], nsel[:, ktt * NE:(ktt + 1) * NE],
                                 v_sb[:, ktt * DV:(ktt + 1) * DV],
                                 start=(kt == 0), stop=(kt == 7))
            nspur = p3.tile([NE, DV], F16, name="nspur", tag="nspur", bufs=1)
            nc.scalar.copy(nspur[:], psn[0:NE, :])
            # --- qmT for all r over this batch: [r][d2c] -> [128, TB] ---
            qmT = []
            for r in range(R):
                pair = []
                for d2c in range(2):
                    qm = p3.tile([128, TB], F16, name="qmT", tag=f"qmT{r}{d2c}", bufs=1)
                    for th in range(2):
                        psq = p3ps.tile([128, 512], F32, name="psqm", tag="ps_misc", bufs=2)
                        for dc in range(2):
                            nc.tensor.matmul(
                                psq[:],
                                wqm_sb[:, dc * 1536 + r * 256 + d2c * 128:
                                       dc * 1536 + r * 256 + d2c * 128 + 128],
                                qT[:, dc * T + b * TB + th * 512:
                                   dc * T + b * TB + th * 512 + 512],
                                start=(dc == 0), stop=(dc == 1))
                        nc.scalar.copy(qm[:, th * 512:(th + 1) * 512], psq[:])
                    pair.append(qm)
                qmT.append(pair)
            for qh in range(2):  # query halves of 512
                for qt in range(4):  # query tiles of 128
                    tt = b * 8 + qh * 4 + qt
                    q0 = qh * 512 + qt * 128
                    ptps = p3ps.tile([128, 1024], F32, name="ptps", tag="ptps", bufs=1)
                    csb = p3.tile([128, NE], F32, name="csb", tag="csb", bufs=2)
                    for r in range(R):
                        krs = 0 if r < 2 else r - 1
                        es_pair = []
                        dsum = []
                        for kc in range(2):
                            sps = p3ps.tile([128, 512], F32, name="sps", tag="sps", bufs=2)
                            for d2c in range(2):
                                nc.tensor.matmul(
                                    sps[:], qmT[r][d2c][:, q0:q0 + 128],
                                    ktm[krs][d2c][:, kc * 512:(kc + 1) * 512],
                                    start=(d2c == 0), stop=(d2c == 1))
                            es = p3.tile([128, 512], F16, name="es", tag="es", bufs=4)
                            dn = p3.tile([128, 1], F32, name="dn", tag="dn", bufs=4)
                            nc.scalar.activation(es[:], sps[:], ACTF.Exp, scale=SCALE,
                                                 accum_out=dn[:])
                            es_pair.append(es)
                            dsum.append(dn)
                        dtot = p3.tile([128, 1], F32, name="dtot", tag="dtot", bufs=2)
                        nc.vector.tensor_tensor(dtot[:], dsum[0][:], dsum[1][:], ALU.add)
                        dinv = p3.tile([128, 1], F32, name="adinv", tag="adinv", bufs=2)
                        nc.vector.reciprocal(dinv[:], dtot[:])
                        cmul = p3.tile([128, 1], F32, name="cmul", tag="cmul", bufs=2)
                        nc.vector.tensor_tensor(cmul[:], dinv[:],
                                                rw_all[:, tt * R + r:tt * R + r + 1],
                                                ALU.mult)
                        if r >= 2:
                            nc.scalar.copy(csb[:, r - 2:r - 1], cmul[:])
                        dcd = p3.tile([128, 128], F16, name="dcd", tag="dcd", bufs=2)
                        nc.vector.tensor_scalar(dcd[:], ident16[:], cmul[:], None, ALU.mult)
                        # combine: P~T[k-tile, q-tile] += expS^T[:,kt]^T @ diag(c)
                        # start only on the first matmul touching each PSUM
                        # bank (start clears has_written for the whole bank).
                        for kt in range(8):
                            nc.tensor.matmul(
                                ptps[:, kt * 128:(kt + 1) * 128],
                                es_pair[kt // 4][:, (kt % 4) * 128:(kt % 4) * 128 + 128],
                                dcd[:], start=(r == 0 and kt % 4 == 0),
                                stop=(r == R - 1))
                    pts = p3.tile([128, 1024], F16, name="pts", tag="pts", bufs=2)
                    nc.scalar.copy(pts[:], ptps[:])
                    # CT: [4, 128] = csb^T
                    psc = p3ps.tile([128, 128], F32, name="psc", tag="ps_ct", bufs=1)
                    nc.tensor.matmul(psc[0:NE, :], csb[:], ident[:], start=True, stop=True)
                    ctb = p3.tile([NE, 128], F16, name="ctb", tag="ctb", bufs=2)
                    nc.scalar.copy(ctb[:], psc[0:NE, :])
                    # AV + spur correction
                    avp = p3ps.tile([128, DV], F32, name="avp", tag="avp", bufs=1)
                    for kt in range(8):
                        ktt = b * 8 + kt
                        nc.tensor.matmul(avp[:], pts[:, kt * 128:(kt + 1) * 128],
                                         v_sb[:, ktt * DV:(ktt + 1) * DV],
                                         start=(kt == 0), stop=False)
                    nc.tensor.matmul(avp[:], ctb[:], nspur[:], start=False, stop=True)
                    nc.vector.tensor_copy(o_acc[:, tt * DV:(tt + 1) * DV], avp[:])
            if b == 0:  # prefetch Wo during second batch's attention
                for i in range(4):
                    nc.sync.dma_start(out=wo_sb[i][:], in_=wo[i * 128:(i + 1) * 128, :])

    # ---------------- phase 4: gate, transpose, output projection ----------------
    with tc.tile_pool(name="p4", bufs=1) as p4, \
         tc.tile_pool(name="p4ps", bufs=1, space="PSUM") as p4ps:
        Xt = [p4.tile([128, T], F16, name=f"xt{i}", tag=f"xt{i}") for i in range(4)]
        for tt in range(16):
            gv = g_sb[:, tt * DV:(tt + 1) * DV]
            sg = p4.tile([128, DV], F16, name="sg", tag="sg", bufs=3)
            nc.scalar.activation(sg[:], gv, ACTF.Sigmoid)
            nc.vector.tensor_tensor(sg[:], sg[:], gv, ALU.mult)  # silu(g)
            xres = p4.tile([128, DV], F16, name="xres", tag="xres", bufs=3)
            nc.vector.tensor_tensor(xres[:], o_acc[:, tt * DV:(tt + 1) * DV], sg[:],
                                    ALU.mult)
            for dvc in range(4):
                pst = p4ps.tile([128, 128], F16, name="pst4", tag="pst4", bufs=2)
                nc.tensor.transpose(pst[:], xres[:, dvc * 128:(dvc + 1) * 128], ident16[:])
                nc.vector.tensor_copy(Xt[dvc][:, tt * 128:(tt + 1) * 128], pst[:])
        for tt in range(16):
            for hb in range(4):
                psf = p4ps.tile([128, 512], F32, name="psf", tag="psf", bufs=2)
                for dvc in range(4):
                    nc.tensor.matmul(psf[:], Xt[dvc][:, tt * 128:(tt + 1) * 128],
                                     wo_sb[dvc][:, hb * 512:(hb + 1) * 512],
                                     start=(dvc == 0), stop=(dvc == 3))
                ost = p4.tile([128, 512], F32, name="ost", tag="ost", bufs=4)
                nc.scalar.copy(ost[:], psf[:])
                nc.sync.dma_start(out=out[tt * 128:(tt + 1) * 128, hb * 512:(hb + 1) * 512],
                                  in_=ost[:])


_PROGRAM = None


def build_program():
    global _PROGRAM
    if _PROGRAM is not None:
        return _PROGRAM
    nc = bacc.Bacc("TRN2", target_bir_lowering=False, debug=False, num_devices=8)
    names = [("wq", [HID, D], F16), ("wk", [HID, D], F16),
             ("wv", [HID, DV], F16), ("wg", [HID, DV], F16),
             ("wqm", [D, D * R], F16),
             ("hsh", [HID, T], BF16), ("hsl", [HID, T], BF16),
             ("wfh", [HID, NE], BF16), ("wfl", [HID, NE], BF16), ("wo", [DV, HID], F16)]
    io = [nc.dram_tensor(n, s, dt, kind="ExternalInput").ap() for n, s, dt in names]
    io.append(nc.dram_tensor("out", [T, HID], F32, kind="ExternalOutput").ap())
    with tile.TileContext(nc) as tc:
        from contextlib import ExitStack as ES
        with ES() as ctx:
            _body(ctx, nc, tc, io)
    nc.compile()
    _PROGRAM = nc
    return nc


def make_in_maps(hidden_states, Wq, Wk, Wv, Wq_exp, Wk_exp, Wgate, Wg, Wo):
    import ml_dtypes
    bf = ml_dtypes.bfloat16
    hs2 = np.asarray(hidden_states, np.float32).reshape(T, HID)
    hsT = np.ascontiguousarray(hs2.T)
    hsh = np.ascontiguousarray(hsT.astype(bf))
    hsl = np.ascontiguousarray((hsT.astype(np.float64) - hsh.astype(np.float64)).astype(bf))
    Wq64 = np.asarray(Wq, np.float64)
    Wg64 = np.asarray(Wgate, np.float64)
    Wqe64 = np.asarray(Wq_exp, np.float64)
    Wke64 = np.asarray(Wk_exp, np.float64)
    in_maps = []
    for c in range(8):
        wfu = Wq64[:, c * D:(c + 1) * D] @ Wg64
        wfh = wfu.astype(bf)
        wfl = (wfu - wfh.astype(np.float64)).astype(bf)
        # M_r = Wq_exp_r @ Wk_exp_r^T : qmT[d2,t] = sum_dq M_r[dq,d2] qT[dq,t]
        wqm = np.empty((D, D * R), np.float16)
        for r in range(R):
            m = Wqe64[c][:, r * D:(r + 1) * D] @ Wke64[c][:, r * D:(r + 1) * D].T
            wqm[:, r * D:(r + 1) * D] = m.astype(np.float16)
        in_maps.append({
            "wq": np.asarray(Wq, np.float16)[:, c * D:(c + 1) * D].copy(),
            "wk": np.asarray(Wk, np.float16)[:, c * D:(c + 1) * D].copy(),
            "wv": np.asarray(Wv, np.float16)[:, c * DV:(c + 1) * DV].copy(),
            "wg": np.asarray(Wg, np.float16)[:, c * DV:(c + 1) * DV].copy(),
            "wqm": wqm,
            "hsh": hsh, "hsl": hsl,
            "wfh": np.ascontiguousarray(wfh), "wfl": np.ascontiguousarray(wfl),
            "wo": np.asarray(Wo, np.float16)[c * DV:(c + 1) * DV, :].copy(),
        })
    return in_maps


def kernel(hidden_states, Wq, Wk, Wv, Wq_exp, Wk_exp, Wgate, Wg, Wo):
    nc = build_program()
    in_maps = make_in_maps(hidden_states, Wq, Wk, Wv, Wq_exp, Wk_exp, Wgate, Wg, Wo)
    res = run_bass_kernel_spmd(nc, in_maps, list(range(8))).results
    out = np.zeros((T, HID), np.float32)
    for c in range(8):
        out += res[c]["out"]
    return out.reshape(2, 1024, HID).astype(np.float32)


# revision 12
# speedup vs baseline: 1.5376x; 1.0759x over previous
"""Trainium2 Bass kernel for MockMobGatedDeltaNetMoE (v3).

Sharding: head-parallel over H=8 heads, one head per NeuronCore.
Each core computes its head's full contribution; the host sums the 8
partial output projections.

v3 = v2 reformulation + fp16 compute path + weights-loaded-once phase 1:
 - Hidden states ship ONLY as an exact bf16 hi/lo pair (router needs exact
   fp32 logits for stable top-2); fp16 hs for the projections is derived
   on-device as hi+lo (one DVE add per tile).
 - All projection/attention matmuls run in fp16 (operand rounding ~5e-4):
   2-byte dtype enables FWL fast weight load; PSUM accumulation stays fp32.
 - Score fusion: S_r = q @ (Wq_exp_r @ Wk_exp_r^T) @ k^T, M_r fused on host.
 - Key masking per routed expert in [token, d] layout, PE-transposed to
   [d, token]; masked keys give exp(0)=1 = the reference denominator.
 - Scores computed transposed [query, key]; exp emits the denominator via
   accum_out.
 - Expert combine in PSUM via diagonal matmuls (transpose + scale +
   accumulate in one instruction per (expert, key-tile)); single
   attention @ V matmul per query tile + rank-4 masked-v correction.
"""

import numpy as np

import concourse.bass as bass
import concourse.bacc as bacc
import concourse.tile as tile
from concourse import mybir
from concourse.bass_utils import run_bass_kernel_spmd

F32 = mybir.dt.float32
F16 = mybir.dt.float16
BF16 = mybir.dt.bfloat16
ALU = mybir.AluOpType
ACTF = mybir.ActivationFunctionType
AX = mybir.AxisListType

H, D, R, NE = 8, 256, 6, 4          # heads, head_dim, experts, routed experts
HID, DV, T = 2048, 512, 2048        # hidden, head_v_dim, b*t tokens
NB = 2                              # batch
TB = T // NB                        # tokens per batch (attention window)
SCALE = 1.0 / 16.0                  # 1/sqrt(D)


def _body(ctx, nc, tc, io):
    wq, wk, wv, wg, wqm, hsh, hsl, wfh, wfl, wo, out = io

    const = ctx.enter_context(tc.tile_pool(name="const", bufs=1))
    pers = ctx.enter_context(tc.tile_pool(name="pers", bufs=1))

    from concourse.masks import make_identity
    ident = const.tile([128, 128], F32, name="ident")
    make_identity(nc, ident)
    ident16 = const.tile([128, 128], F16, name="ident16")
    nc.vector.tensor_copy(ident16[:], ident[:])
    wfh_sb = const.tile([128, 64], BF16, name="wfh_sb")
    wfl_sb = const.tile([128, 64], BF16, name="wfl_sb")
    for hc in range(16):
        nc.sync.dma_start(out=wfh_sb[:, hc * 4:(hc + 1) * 4],
                          in_=wfh[hc * 128:(hc + 1) * 128, :])
        nc.sync.dma_start(out=wfl_sb[:, hc * 4:(hc + 1) * 4],
                          in_=wfl[hc * 128:(hc + 1) * 128, :])
    logit_sb = pers.tile([128, 64], F32, name="logit_sb")

    # persistent tensors (fp16)
    qT = pers.tile([128, 2 * T], F16, name="qT")         # [d-chunk, token]
    k_sb = pers.tile([128, 16 * D], F16, name="k_sb")    # [token-tile, d]
    v_sb = pers.tile([128, 16 * DV], F16, name="v_sb")   # [token-tile, dv]
    g_sb = pers.tile([128, 16 * DV], F16, name="g_sb")   # [token-tile, dv]
    wqm_sb = pers.tile([128, 2 * 1536], F16, name="wqm_sb")
    rw_all = pers.tile([128, 16 * R], F32, name="rw_all")
    msel = pers.tile([128, 16 * NE], F32, name="msel")   # top-2 mask (1/0)
    nsel = pers.tile([128, 16 * NE], F16, name="nsel")   # sel - 1 (0/-1)
    for dc in range(2):
        nc.sync.dma_start(out=wqm_sb[:, dc * 1536:(dc + 1) * 1536],
                          in_=wqm[dc * 128:(dc + 1) * 128, :])

    # ---------------- phase 1: projections (weights loaded once) ----------------
    with tc.tile_pool(name="p1w", bufs=1) as p1w, \
         tc.tile_pool(name="p1", bufs=1) as p1, \
         tc.tile_pool(name="p1ps", bufs=1, space="PSUM") as p1ps:
        # interleave hidden-state and weight DMAs per hid-chunk so the first
        # matmul (needs hst[0] + wq_sb[0]) can start after ~2 transfers, not
        # after the whole weight block.
        wq_sb, wk_sb, wv_sb, wg_sb = [], [], [], []
        hih0, hil0, hst0 = [], [], []
        for hc in range(16):
            h1 = p1.tile([128, 512], BF16, name="hih", tag="hih", bufs=17)
            nc.sync.dma_start(out=h1[:], in_=hsh[hc * 128:(hc + 1) * 128, 0:512])
            hih0.append(h1)
            h2 = p1.tile([128, 512], BF16, name="hil", tag="hil", bufs=17)
            nc.sync.dma_start(out=h2[:], in_=hsl[hc * 128:(hc + 1) * 128, 0:512])
            hil0.append(h2)
            h3 = p1.tile([128, 512], F16, name="hst", tag="hst", bufs=17)
            nc.vector.tensor_tensor(h3[:], h1[:], h2[:], ALU.add)  # fp16 hs
            hst0.append(h3)
            for lst, src, wdt, nm in ((wq_sb, wq, 256, "wqsb"), (wk_sb, wk, 256, "wksb"),
                                      (wv_sb, wv, 512, "wvsb"), (wg_sb, wg, 512, "wgsb")):
                w1 = p1w.tile([128, wdt], F16, name=nm, tag=f"{nm}{hc}")
                nc.sync.dma_start(out=w1[:], in_=src[hc * 128:(hc + 1) * 128, :])
                lst.append(w1)
        for tb in range(4):  # token blocks of 512
            t0 = tb * 512
            if tb == 0:
                hih, hil, hst = hih0, hil0, hst0
            else:
                hih, hil, hst = [], [], []
                for hc in range(16):
                    h1 = p1.tile([128, 512], BF16, name="hih", tag="hih", bufs=17)
                    nc.sync.dma_start(out=h1[:], in_=hsh[hc * 128:(hc + 1) * 128, t0:t0 + 512])
                    hih.append(h1)
                    h2 = p1.tile([128, 512], BF16, name="hil", tag="hil", bufs=17)
                    nc.sync.dma_start(out=h2[:], in_=hsl[hc * 128:(hc + 1) * 128, t0:t0 + 512])
                    hil.append(h2)
                    h3 = p1.tile([128, 512], F16, name="hst", tag="hst", bufs=17)
                    nc.vector.tensor_tensor(h3[:], h1[:], h2[:], ALU.add)  # fp16 hs
                    hst.append(h3)
            # q projection -> qT [d-chunk, token]
            for f in range(2):
                ps = p1ps.tile([128, 512], F32, name="psq", tag="psq", bufs=2)
                for hc in range(16):
                    nc.tensor.matmul(ps[:], wq_sb[hc][:, f * 128:(f + 1) * 128],
                                     hst[hc][:], start=(hc == 0), stop=(hc == 15))
                nc.scalar.copy(qT[:, f * T + t0:f * T + t0 + 512], ps[:])
            # k/v/g: one shared-stationary pass per token tile
            for tt4 in range(4):
                tt = tb * 4 + tt4
                ps = p1ps.tile([128, 1536], F32, name="pskvg", tag="pskvg", bufs=2)
                for hc in range(16):
                    st_ap = hst[hc][:, tt4 * 128:(tt4 + 1) * 128]
                    nc.tensor.matmul(ps[:, 0:256], st_ap, wk_sb[hc][:],
                                     start=(hc == 0), stop=(hc == 15))
                    nc.tensor.matmul(ps[:, 512:1024], st_ap, wv_sb[hc][:],
                                     start=(hc == 0), stop=(hc == 15))
                    nc.tensor.matmul(ps[:, 1024:1536], st_ap, wg_sb[hc][:],
                                     start=(hc == 0), stop=(hc == 15))
                nc.scalar.copy(k_sb[:, tt * D:(tt + 1) * D], ps[:, 0:256])
                nc.scalar.copy(v_sb[:, tt * DV:(tt + 1) * DV], ps[:, 512:1024])
                nc.vector.tensor_copy(g_sb[:, tt * DV:(tt + 1) * DV], ps[:, 1024:1536])
            # routing logits: 3-term bf16 split-GEMM (exact products, fp32 accum)
            for tl in range(4):
                tt = tb * 4 + tl
                psr = p1ps.tile([128, 4], F32, name="psr", tag="psq", bufs=2)
                n_mm = 0
                for aa, bb in ((hih, wfh_sb), (hih, wfl_sb), (hil, wfh_sb)):
                    for hc in range(16):
                        nc.tensor.matmul(psr[:],
                                         aa[hc][:, tl * 128:(tl + 1) * 128],
                                         bb[:, hc * 4:(hc + 1) * 4],
                                         start=(n_mm == 0), stop=(n_mm == 47))
                        n_mm += 1
                nc.scalar.copy(logit_sb[:, tt * 4:(tt + 1) * 4], psr[:])

    # ---------------- phase 2: routing ----------------
    nc.vector.memset(rw_all[:], 0.25)
    with tc.tile_pool(name="p2", bufs=4) as p2:
        for tt in range(16):
            lg = logit_sb[:, tt * 4:(tt + 1) * 4]
            s = p2.tile([128, 4], F32, name="s")
            nc.scalar.activation(s[:], lg, ACTF.Exp)
            m1 = p2.tile([128, 1], F32, name="m1")
            nc.vector.tensor_reduce(m1[:], lg, axis=AX.X, op=ALU.max)
            eq = p2.tile([128, 4], F32, name="eq")
            nc.vector.tensor_scalar(eq[:], lg, m1[:], None, ALU.is_ge)
            sm = p2.tile([128, 4], F32, name="sm")
            nc.vector.scalar_tensor_tensor(sm[:], eq[:], -1e30, lg, ALU.mult, ALU.add)
            m2 = p2.tile([128, 1], F32, name="m2")
            nc.vector.tensor_reduce(m2[:], sm[:], axis=AX.X, op=ALU.max)
            sel = p2.tile([128, 4], F32, name="sel")
            nc.vector.tensor_scalar(sel[:], lg, m2[:], None, ALU.is_ge)
            w4 = p2.tile([128, 4], F32, name="w4")
            nc.vector.tensor_tensor(w4[:], s[:], sel[:], ALU.mult)
            den = p2.tile([128, 1], F32, name="den")
            nc.vector.tensor_reduce(den[:], w4[:], axis=AX.X, op=ALU.add)
            dinv = p2.tile([128, 1], F32, name="dinv")
            nc.vector.reciprocal(dinv[:], den[:])
            nc.vector.tensor_scalar(rw_all[:, tt * R + 2:tt * R + 6], w4[:], dinv[:], 0.5,
                                    ALU.mult, ALU.mult)
            nc.scalar.copy(msel[:, tt * NE:(tt + 1) * NE], sel[:])
            nc.vector.tensor_scalar(nsel[:, tt * NE:(tt + 1) * NE], sel[:], 1.0, -1.0,
                                    ALU.mult, ALU.add)

    # ---------------- phase 3: expert attention (combined) ----------------
    pers3 = ctx.enter_context(tc.tile_pool(name="pers3", bufs=1))
    o_acc = pers3.tile([128, 16 * DV], F32, name="o_acc")
    wo_sb = [pers3.tile([128, HID], F16, name=f"wo_sb{i}") for i in range(4)]
    with tc.tile_pool(name="p3", bufs=1) as p3, \
         tc.tile_pool(name="p3ps", bufs=1, space="PSUM") as p3ps:
        for b in range(NB):
            # --- kTm: masked-transposed keys, 5 sets (shared + 4 routed) ---
            ktm = []  # [set][dc] -> [128, TB] fp16
            for rs in range(5):
                pair = [p3.tile([128, TB], F16, name="ktm", tag=f"ktm{rs}{dc}", bufs=2)
                        for dc in range(2)]
                ktm.append(pair)
                for kt in range(8):
                    ktt = b * 8 + kt
                    if rs == 0:
                        src = k_sb[:, ktt * D:(ktt + 1) * D]
                    else:
                        km = p3.tile([128, D], F16, name="km", tag="km", bufs=2)
                        nc.vector.tensor_scalar(
                            km[:], k_sb[:, ktt * D:(ktt + 1) * D],
                            msel[:, ktt * NE + (rs - 1):ktt * NE + rs], None, ALU.mult)
                        src = km[:]
                    for dc in range(2):
                        pst = p3ps.tile([128, 128], F16, name="pst", tag="ps_misc", bufs=2)
                        nc.tensor.transpose(pst[:], src[:, dc * 128:(dc + 1) * 128],
                                            ident16[:])
                        nc.vector.tensor_copy(pair[dc][:, kt * 128:(kt + 1) * 128], pst[:])
            # --- nspur_b[r', :] = -sum_{masked k} v[k, :]  (rank-4) ---
            psn = p3ps.tile([128, DV], F32, name="psn", tag="ps_misc", bufs=2)
            for kt in range(8):
                ktt = b * 8 + kt
                nc.tensor.matmul(psn[0:NE, :], nsel[:, ktt * NE:(ktt + 1) * NE],
                                 v_sb[:, ktt * DV:(ktt + 1) * DV],
                                 start=(kt == 0), stop=(kt == 7))
            nspur = p3.tile([NE, DV], F16, name="nspur", tag="nspur", bufs=2)
            nc.scalar.copy(nspur[:], psn[0:NE, :])
            # --- qmT for all r over this batch: [r][d2c] -> [128, TB] ---
            qmT = []
            for r in range(R):
                pair = []
                for d2c in range(2):
                    qm = p3.tile([128, TB], F16, name="qmT", tag=f"qmT{r}{d2c}", bufs=2)
                    for th in range(2):
                        psq = p3ps.tile([128, 512], F32, name="psqm", tag="ps_misc", bufs=2)
                        for dc in range(2):
                            nc.tensor.matmul(
                                psq[:],
                                wqm_sb[:, dc * 1536 + r * 256 + d2c * 128:
                                       dc * 1536 + r * 256 + d2c * 128 + 128],
                                qT[:, dc * T + b * TB + th * 512:
                                   dc * T + b * TB + th * 512 + 512],
                                start=(dc == 0), stop=(dc == 1))
                        nc.vector.tensor_copy(qm[:, th * 512:(th + 1) * 512], psq[:])
                    pair.append(qm)
                qmT.append(pair)
            for qh in range(2):  # query halves of 512
                for qt in range(4):  # query tiles of 128
                    tt = b * 8 + qh * 4 + qt
                    q0 = qh * 512 + qt * 128
                    ptps = p3ps.tile([128, 1024], F32, name="ptps", tag="ptps", bufs=1)
                    csb = p3.tile([128, NE], F32, name="csb", tag="csb", bufs=2)
                    for r in range(R):
                        krs = 0 if r < 2 else r - 1
                        es_pair = []
                        dsum = []
                        for kc in range(2):
                            sps = p3ps.tile([128, 512], F32, name="sps", tag="sps", bufs=2)
                            for d2c in range(2):
                                nc.tensor.matmul(
                                    sps[:], qmT[r][d2c][:, q0:q0 + 128],
                                    ktm[krs][d2c][:, kc * 512:(kc + 1) * 512],
                                    start=(d2c == 0), stop=(d2c == 1))
                            es = p3.tile([128, 512], F16, name="es", tag="es", bufs=4)
                            dn = p3.tile([128, 1], F32, name="dn", tag="dn", bufs=4)
                            nc.scalar.activation(es[:], sps[:], ACTF.Exp, scale=SCALE,
                                                 accum_out=dn[:])
                            es_pair.append(es)
                            dsum.append(dn)
                        dtot = p3.tile([128, 1], F32, name="dtot", tag="dtot", bufs=2)
                        nc.vector.tensor_tensor(dtot[:], dsum[0][:], dsum[1][:], ALU.add)
                        dinv = p3.tile([128, 1], F32, name="adinv", tag="adinv", bufs=2)
                        nc.vector.reciprocal(dinv[:], dtot[:])
                        cmul = p3.tile([128, 1], F32, name="cmul", tag="cmul", bufs=2)
                        nc.vector.tensor_tensor(cmul[:], dinv[:],
                                                rw_all[:, tt * R + r:tt * R + r + 1],
                                                ALU.mult)
                        if r >= 2:
                            nc.vector.tensor_copy(csb[:, r - 2:r - 1], cmul[:])
                        dcd = p3.tile([128, 128], F16, name="dcd", tag="dcd", bufs=2)
                        nc.vector.tensor_scalar(dcd[:], ident16[:], cmul[:], None, ALU.mult)
                        # combine: P~T[k-tile, q-tile] += expS^T[:,kt]^T @ diag(c)
                        # start only on the first matmul touching each PSUM
                        # bank (start clears has_written for the whole bank).
                        for kt in range(8):
                            nc.tensor.matmul(
                                ptps[:, kt * 128:(kt + 1) * 128],
                                es_pair[kt // 4][:, (kt % 4) * 128:(kt % 4) * 128 + 128],
                                dcd[:], start=(r == 0 and kt % 4 == 0),
                                stop=(r == R - 1))
                    pts = p3.tile([128, 1024], F16, name="pts", tag="pts", bufs=2)
                    nc.vector.tensor_copy(pts[:], ptps[:])
                    # CT: [4, 128] = csb^T
                    psc = p3ps.tile([128, 128], F32, name="psc", tag="ps_ct", bufs=1)
                    nc.tensor.matmul(psc[0:NE, :], csb[:], ident[:], start=True, stop=True)
                    ctb = p3.tile([NE, 128], F16, name="ctb", tag="ctb", bufs=2)
                    nc.scalar.copy(ctb[:], psc[0:NE, :])
                    # AV + spur correction
                    avp = p3ps.tile([128, DV], F32, name="avp", tag="avp", bufs=1)
                    for kt in range(8):
                        ktt = b * 8 + kt
                        nc.tensor.matmul(avp[:], pts[:, kt * 128:(kt + 1) * 128],
                                         v_sb[:, ktt * DV:(ktt + 1) * DV],
                                         start=(kt == 0), stop=False)
                    nc.tensor.matmul(avp[:], ctb[:], nspur[:], start=False, stop=True)
                    nc.vector.tensor_copy(o_acc[:, tt * DV:(tt + 1) * DV], avp[:])
            if b == 0:  # prefetch Wo during second batch's attention
                for i in range(4):
                    nc.sync.dma_start(out=wo_sb[i][:], in_=wo[i * 128:(i + 1) * 128, :])

    # ---------------- phase 4: gate, transpose, output projection ----------------
    with tc.tile_pool(name="p4", bufs=1) as p4, \
         tc.tile_pool(name="p4ps", bufs=1, space="PSUM") as p4ps:
        Xt = [p4.tile([128, T], F16, name=f"xt{i}", tag=f"xt{i}") for i in range(4)]
        for tt in range(16):
            gv = g_sb[:, tt * DV:(tt + 1) * DV]
            sg = p4.tile([128, DV], F16, name="sg", tag="sg", bufs=3)
            nc.scalar.activation(sg[:], gv, ACTF.Sigmoid)
            nc.vector.tensor_tensor(sg[:], sg[:], gv, ALU.mult)  # silu(g)
            xres = p4.tile([128, DV], F16, name="xres", tag="xres", bufs=3)
            nc.vector.tensor_tensor(xres[:], o_acc[:, tt * DV:(tt + 1) * DV], sg[:],
                                    ALU.mult)
            for dvc in range(4):
                pst = p4ps.tile([128, 128], F16, name="pst4", tag="pst4", bufs=2)
                nc.tensor.transpose(pst[:], xres[:, dvc * 128:(dvc + 1) * 128], ident16[:])
                nc.vector.tensor_copy(Xt[dvc][:, tt * 128:(tt + 1) * 128], pst[:])
        for tt in range(16):
            for hb in range(4):
                psf = p4ps.tile([128, 512], F32, name="psf", tag="psf", bufs=2)
                for dvc in range(4):
                    nc.tensor.matmul(psf[:], Xt[dvc][:, tt * 128:(tt + 1) * 128],
                                     wo_sb[dvc][:, hb * 512:(hb + 1) * 512],
                                     start=(dvc == 0), stop=(dvc == 3))
                ost = p4.tile([128, 512], F16, name="ost", tag="ost", bufs=4)
                nc.scalar.copy(ost[:], psf[:])
                nc.sync.dma_start(out=out[tt * 128:(tt + 1) * 128, hb * 512:(hb + 1) * 512],
                                  in_=ost[:])


_PROGRAM = None


def build_program():
    global _PROGRAM
    if _PROGRAM is not None:
        return _PROGRAM
    nc = bacc.Bacc("TRN2", target_bir_lowering=False, debug=False, num_devices=8)
    names = [("wq", [HID, D], F16), ("wk", [HID, D], F16),
             ("wv", [HID, DV], F16), ("wg", [HID, DV], F16),
             ("wqm", [D, D * R], F16),
             ("hsh", [HID, T], BF16), ("hsl", [HID, T], BF16),
             ("wfh", [HID, NE], BF16), ("wfl", [HID, NE], BF16), ("wo", [DV, HID], F16)]
    io = [nc.dram_tensor(n, s, dt, kind="ExternalInput").ap() for n, s, dt in names]
    io.append(nc.dram_tensor("out", [T, HID], F16, kind="ExternalOutput").ap())
    with tile.TileContext(nc) as tc:
        from contextlib import ExitStack as ES
        with ES() as ctx:
            _body(ctx, nc, tc, io)
    nc.compile()
    _PROGRAM = nc
    return nc


def make_in_maps(hidden_states, Wq, Wk, Wv, Wq_exp, Wk_exp, Wgate, Wg, Wo):
    import ml_dtypes
    bf = ml_dtypes.bfloat16
    hs2 = np.asarray(hidden_states, np.float32).reshape(T, HID)
    hsT = np.ascontiguousarray(hs2.T)
    hsh = np.ascontiguousarray(hsT.astype(bf))
    hsl = np.ascontiguousarray((hsT.astype(np.float64) - hsh.astype(np.float64)).astype(bf))
    Wq64 = np.asarray(Wq, np.float64)
    Wg64 = np.asarray(Wgate, np.float64)
    Wqe64 = np.asarray(Wq_exp, np.float64)
    Wke64 = np.asarray(Wk_exp, np.float64)
    in_maps = []
    for c in range(8):
        wfu = Wq64[:, c * D:(c + 1) * D] @ Wg64
        wfh = wfu.astype(bf)
        wfl = (wfu - wfh.astype(np.float64)).astype(bf)
        # M_r = Wq_exp_r @ Wk_exp_r^T : qmT[d2,t] = sum_dq M_r[dq,d2] qT[dq,t]
        wqm = np.empty((D, D * R), np.float16)
        for r in range(R):
            m = Wqe64[c][:, r * D:(r + 1) * D] @ Wke64[c][:, r * D:(r + 1) * D].T
            wqm[:, r * D:(r + 1) * D] = m.astype(np.float16)
        in_maps.append({
            "wq": np.asarray(Wq, np.float16)[:, c * D:(c + 1) * D].copy(),
            "wk": np.asarray(Wk, np.float16)[:, c * D:(c + 1) * D].copy(),
            "wv": np.asarray(Wv, np.float16)[:, c * DV:(c + 1) * DV].copy(),
            "wg": np.asarray(Wg, np.float16)[:, c * DV:(c + 1) * DV].copy(),
            "wqm": wqm,
            "hsh": hsh, "hsl": hsl,
            "wfh": np.ascontiguousarray(wfh), "wfl": np.ascontiguousarray(wfl),
            "wo": np.asarray(Wo, np.float16)[c * DV:(c + 1) * DV, :].copy(),
        })
    return in_maps


def kernel(hidden_states, Wq, Wk, Wv, Wq_exp, Wk_exp, Wgate, Wg, Wo):
    nc = build_program()
    in_maps = make_in_maps(hidden_states, Wq, Wk, Wv, Wq_exp, Wk_exp, Wgate, Wg, Wo)
    res = run_bass_kernel_spmd(nc, in_maps, list(range(8))).results
    out = np.zeros((T, HID), np.float32)
    for c in range(8):
        out += res[c]["out"].astype(np.float32)
    return out.reshape(2, 1024, HID).astype(np.float32)
